# revision 28
# baseline (speedup 1.0000x reference)
"""Trainium2 Bass kernel for nn_NeuralMirrorModule (Bregman divergence loss).

Math: the reference's per-element computation collapses to
    div(y,y0) = S(y) - S(y0) - S'(y0)*(y-y0) + a/2*(y-y0)^2
                + c*(y*(ln ys - ln y0s) - (y-y0))
with S(t) = sum_j v_j * H_j(t) a fixed univariate function of t in [0,1)
determined by the 126 (v,w,b) parameters.  On the host we fit S and S'
with Chebyshev polynomials (fp64); the approximation error sits below
the reference's own fp32 noise floor (~2.9e-6 abs vs absmax ~0.096).
The device evaluates, per element:
    out = P(y) + c*y*ln(y) - U(y0) - (Q(y0) + c*ln(y0s))*y
where P/U/Q are polynomials evaluated by Horner chains in a normalized
variable chi = lam*(2t-1) chosen so the leading coefficient is +-1 -- the
first fused op then consumes 4 coefficients and no chain-start op is
needed.  Horner runs 3 steps per DVE instruction via a custom fused op
(((acc+c1)*x+c2)*x+c3)*x, cody-waite style; the combine tail is 5 more
fused DVE ops; the two logs run on the ACT engine (with the EPS_PROB
clamp folded into ACT as relu(y0-eps)+eps).

Sharding: flat 2M elements -> 8 cores x [128, 2048]; params replicated
(polynomial coefficients baked as instruction immediates).  No
communication.
"""

import numpy as np

NCORES = 8
P_DIM, F_DIM = 128, 2048
PER_CORE = P_DIM * F_DIM          # 262144
DEG_S, DEG_SPU, DEG_SPQ = 7, 9, 10
EPS = 1e-3                        # log clamp eps (activation group 4)
EPS_PROB = 1e-10
NG = 21
ONE_THIRD = 1.0 / 3.0

# --------------------------------------------------------------------------- #
# host-side math: collapse the 126-neuron Bregman potential to polynomials
# --------------------------------------------------------------------------- #

def _act(u, g):
    if g == 0: return u ** 3
    if g == 1: return u ** 2
    if g == 2: return np.sqrt(np.maximum(u, 0.0))
    if g == 3: return np.power(np.maximum(u, 0.0), ONE_THIRD)
    if g == 4: return np.log(np.maximum(u, 0.0) + EPS)
    return np.exp(u)


def _prim(u, ws, g):
    if g == 0: return u ** 4 / (4.0 * ws)
    if g == 1: return u ** 3 / (3.0 * ws)
    if g == 2: return (2.0 / 3.0) * np.power(np.maximum(u, 0.0), 1.5) / ws
    if g == 3: return 0.75 * np.power(np.maximum(u, 0.0), 4.0 / 3.0) / ws
    if g == 4:
        us = np.maximum(u, 0.0) + EPS
        return (us * np.log(us) - us) / ws
    return np.exp(u) / ws


def _norm_chain(C):
    """Lead-1 normalization: poly sum C[k] x^k (x = 2t-1) -> chain in
    chi = lam*x with coefficients cp (cp[d] = +1), overall sign."""
    C = np.asarray(C, dtype=np.float64)
    d = len(C) - 1
    if abs(C[d]) < 1e-12 * max(1e-300, np.abs(C).max()):
        # degenerate leading coeff: nudge it; changes the poly negligibly
        C = C.copy()
        C[d] = 1e-12 * max(1e-300, np.abs(C).max()) or 1e-30
    lam = abs(C[d]) ** (1.0 / d)
    sign = 1.0 if C[d] > 0 else -1.0
    cp = np.array([C[k] / lam ** k for k in range(d + 1)]) * sign
    return dict(lam=float(lam), sign=sign, cp=[float(x) for x in cp])


def _gen_coeffs(v, w, b, a, c):
    """Fit S, S' on [0,1]; return normalized device chains and scalars."""
    import numpy.polynomial.chebyshev as Ch
    import numpy.polynomial.polynomial as Pn

    v = v.astype(np.float64); w = w.astype(np.float64); b = b.astype(np.float64)
    a = float(a); c = float(c)

    def S_of(t):
        out = np.zeros_like(t)
        for g in range(6):
            for j in range(g * NG, (g + 1) * NG):
                u = w[j] * t + b[j]
                if abs(w[j]) < 1e-12:       # degenerate branch of the reference
                    out += v[j] * _act(u, g) * t
                else:
                    out += v[j] * _prim(u, w[j], g)
        return out

    def Sp_of(t):
        out = np.zeros_like(t)
        for g in range(6):
            for j in range(g * NG, (g + 1) * NG):
                out += v[j] * _act(w[j] * t + b[j], g)
        return out

    M = 3000
    xn = np.cos(np.pi * (np.arange(M) + 0.5) / M)
    tn = 0.5 * (xn + 1.0)
    S0 = S_of(np.zeros(1))[0]
    Sv = S_of(tn) - S0
    Spv = Sp_of(tn)
    ps = Ch.cheb2poly(Ch.chebfit(xn, Sv, DEG_S))       # S~ fit, in x = 2t-1
    ppu = Ch.cheb2poly(Ch.chebfit(xn, Spv, DEG_SPU))   # S~' fit for U
    ppq = Ch.cheb2poly(Ch.chebfit(xn, Spv, DEG_SPQ))   # S~' fit for Q

    # P(t) = S~(t) + (a/2)t^2 - c t          [evaluated at y]
    P = ps.copy()
    P[0] += a / 8 - c / 2; P[1] += a / 4 - c / 2; P[2] += a / 8
    # U(t) = S~(t) - t S~'(t) - (a/2)t^2 - c t   [evaluated at y0]
    U = Pn.polysub(ps, Pn.polymul(np.array([0.5, 0.5]), ppu))
    U[0] += -(a / 8) - c / 2; U[1] += -(a / 4) - c / 2; U[2] += -(a / 8)
    # Q(t) = S~'(t) + a t                    [evaluated at y0]
    Q = ppq.copy()
    Q[0] += a / 2; Q[1] += a / 2

    return dict(
        P=_norm_chain(P),
        negU=_norm_chain(-U),
        Q=_norm_chain(Q),
        K0=float(P[0] - U[0]),
        q0=float(Q[0]),
        c=c,
    )

# --------------------------------------------------------------------------- #
# custom DVE ops
# --------------------------------------------------------------------------- #

_OPS_CACHE = {}


def _register_dve_ops():
    """Register fused DVE ops in concourse.dve_ops (runtime append, per the
    documented extension API). Idempotent."""
    if _OPS_CACHE:
        return _OPS_CACHE
    import concourse.dve_ops as D
    from concourse.dve_spec import Spec, Src0, Src1, C0, C1, C2, lower
    from concourse.dve_spec import _has_src1
    from concourse.dve_uop import DveOpSpec

    def make(name, body, ref):
        for op in D.OPS:
            if op.name == name:
                return op
        spec = Spec(body=body, reference=ref)
        shas = {}
        for ver in ("v3", "v4"):
            s = DveOpSpec(name=name, opcode=1, uops=lower(spec, ver=ver),
                          rd1_en=_has_src1(spec))
            shas[ver] = s.sha(ver)
        op = D.DveOp(name, spec, subdim=False, uops_sha=shas)
        D.OPS.append(op)
        row = D._CUSTOM_DVE_ROW_BASE + D.OPS.index(op)
        assert row < 0x20, "custom DVE row overflow"
        D._SUB_OPCODE_FOR_NAME[name] = row
        D.CUSTOM_DVE_SPECS[name] = spec
        return op

    f32 = np.float32
    _OPS_CACHE["h3"] = make(
        "HORNER3_ANT",
        (((Src0 + C0) * Src1 + C1) * Src1 + C2) * Src1,
        lambda in0, in1, s0, s1, imm2: (
            ((((in0.astype(f32) + f32(s0)) * in1 + f32(s1)) * in1 + f32(imm2)) * in1)
        ).astype(f32),
    )
    _OPS_CACHE["h2"] = make(
        "HORNER2_ANT",
        ((Src0 + C0) * Src1 + C1) * Src1,
        lambda in0, in1, s0, s1, imm2: (
            ((in0.astype(f32) + f32(s0)) * in1 + f32(s1)) * in1
        ).astype(f32),
    )
    # t2 = (ly0*c + Qacc) + q0   /  minus variant for sign-flipped Q chains
    _OPS_CACHE["logmix_p"] = make(
        "LOGMIXP_ANT",
        (Src0 * C0 + Src1) + C1,
        lambda in0, in1, s0, s1, imm2: (
            (in0.astype(f32) * f32(s0) + in1) + f32(s1)
        ).astype(f32),
    )
    _OPS_CACHE["logmix_m"] = make(
        "LOGMIXM_ANT",
        (Src0 * C0 - Src1) + C1,
        lambda in0, in1, s0, s1, imm2: (
            (in0.astype(f32) * f32(s0) - in1) + f32(s1)
        ).astype(f32),
    )
    # z = ly*c - t2
    _OPS_CACHE["axmy"] = make(
        "AXMY_ANT",
        Src0 * C0 - Src1,
        lambda in0, in1, s0, s1, imm2: (
            in0.astype(f32) * f32(s0) - in1
        ).astype(f32),
    )
    # w = z*y + K0
    _OPS_CACHE["muladd"] = make(
        "MULADD_ANT",
        Src0 * Src1 + C0,
        lambda in0, in1, s0, s1, imm2: (
            in0.astype(f32) * in1 + f32(s0)
        ).astype(f32),
    )
    return _OPS_CACHE

# --------------------------------------------------------------------------- #
# bass program
# --------------------------------------------------------------------------- #


def _emit_norm_chain(nc, acc, chi, ch, h3, h2, out_slices=None):
    """Lead-1 zero-const Horner: acc <- sign * sum_{k>=1} C[k] x^k, where the
    chain runs in chi (= lam*x) with normalized coeffs ch['cp'] (cp[d]=1).
    First fused op reads chi for both streams (no chain-start op)."""
    import concourse.mybir as mybir
    cp = ch["cp"]
    d = len(cp) - 1
    assert d >= 4
    nc.vector._custom_dve(
        h3, out=acc[:], in0=chi[:], in1=chi[:],
        s0=cp[d - 1], s1=cp[d - 2], imm2=cp[d - 3])
    ks = list(range(d - 4, 0, -1))
    i = 0
    while i < len(ks):
        left = len(ks) - i
        if left >= 3:
            nc.vector._custom_dve(
                h3, out=acc[:], in0=acc[:], in1=chi[:],
                s0=cp[ks[i]], s1=cp[ks[i + 1]], imm2=cp[ks[i + 2]])
            i += 3
        elif left == 2:
            nc.vector._custom_dve(
                h2, out=acc[:], in0=acc[:], in1=chi[:],
                s0=cp[ks[i]], s1=cp[ks[i + 1]])
            i += 2
        else:
            nc.vector.scalar_tensor_tensor(
                acc[:], acc[:], cp[ks[i]], chi[:],
                mybir.AluOpType.add, mybir.AluOpType.mult)
            i += 1


def _build_nc(co, debug_taps=()):
    from contextlib import ExitStack
    import concourse.bass as bass
    import concourse.mybir as mybir

    ops = _register_dve_ops()
    h3, h2 = ops["h3"], ops["h2"]
    f32 = mybir.dt.float32
    ALU = mybir.AluOpType
    AF = mybir.ActivationFunctionType
    HF = F_DIM // 2

    nc = bass.Bass()
    y_in = nc.declare_dram_parameter("y_in", [P_DIM, F_DIM], f32, isOutput=False)
    y0_in = nc.declare_dram_parameter("y0_in", [P_DIM, F_DIM], f32, isOutput=False)
    eps_in = nc.declare_dram_parameter("eps_in", [P_DIM, 2], f32, isOutput=False)
    out_d = nc.declare_dram_parameter("out", [P_DIM, F_DIM], f32, isOutput=True)
    dbg_d = {n: nc.declare_dram_parameter("dbg_" + n, [P_DIM, F_DIM], f32, isOutput=True)
             for n in debug_taps}

    sP, sU, sQ = co["P"]["sign"], co["negU"]["sign"], co["Q"]["sign"]
    cc = co["c"]

    with ExitStack() as es:
        def tile(name):
            return es.enter_context(nc.sbuf_tensor(name, [P_DIM, F_DIM], f32))

        ty, ty0 = tile("ty"), tile("ty0")
        chP, chU, chQ, tr = tile("chP"), tile("chU"), tile("chQ"), tile("tr")
        ly, ly0 = tile("ly"), tile("ly0")
        Pacc, nUacc, Qacc = tile("Pacc"), tile("nUacc"), tile("Qacc")
        t2, z, w, s0, res = tile("t2"), tile("z"), tile("w"), tile("s0"), tile("res")
        bias_t = es.enter_context(nc.sbuf_tensor("bias_t", [P_DIM, 2], f32))

        s_in = es.enter_context(nc.semaphore("s_in"))
        s_ing = es.enter_context(nc.semaphore("s_ing"))
        s_act = es.enter_context(nc.semaphore("s_act"))
        s_done = es.enter_context(nc.semaphore("s_done"))
        s_out = es.enter_context(nc.semaphore("s_out"))
        s_out2 = es.enter_context(nc.semaphore("s_out2"))
        block = es.enter_context(nc.Block(no_gpsimd_drain=True))

        tiles_by_name = dict(ty=ty, ty0=ty0, chP=chP, chU=chU, chQ=chQ, tr=tr,
                             ly=ly, ly0=ly0, Pacc=Pacc, nUacc=nUacc, Qacc=Qacc,
                             t2=t2, z=z, w=w, s0=s0, res=res)

        @block.sync
        def _(sync):
            # single whole-tile DMAs: one InstDMACopy already fans out across
            # all 16 SDMA engines; splitting across rings just contends
            sync.dma_start(out=ty0[:], in_=y0_in[:]).then_inc(s_in, 16)
            sync.dma_start(out=ty[:], in_=y_in[:]).then_inc(s_in, 16)
            sync.wait_ge(s_done, 1)
            # no completion wait: the Block-exit Drain on SP drains its HWDGE
            # ring, which implies the output DMA has landed
            sync.dma_start(out=out_d[:, :HF], in_=res[:, :HF]).then_inc(s_out, 16)
            for n in debug_taps:
                sync.dma_start(out=dbg_d[n][:], in_=tiles_by_name[n][:]).then_inc(s_out, 16)

        @block.scalar
        def _(scalar):
            # eps biases ride ACT's own HWDGE ring (tiny)
            scalar.dma_start(out=bias_t[:], in_=eps_in[:]).then_inc(s_ing, 16)
            scalar.wait_ge(s_in, 16)
            scalar.wait_ge(s_ing, 16)
            # ln(max(y0, eps)) == ln(relu(y0 - eps) + eps), all on ACT
            nc.scalar.activation(tr[:], ty0[:], AF.Relu, bias=bias_t[:, 0:1])
            nc.scalar.activation(ly0[:], tr[:], AF.Ln, bias=bias_t[:, 1:2]).then_inc(s_act, 1)
            scalar.wait_ge(s_in, 32)
            nc.scalar.activation(ly[:], ty[:], AF.Ln).then_inc(s_act, 1)
            scalar.wait_ge(s_done, 2)
            scalar.dma_start(out=out_d[:, HF:], in_=res[:, HF:]).then_inc(s_out2, 16)

        @block.vector
        def _(vector):
            vector.wait_ge(s_in, 16)
            # chi variables; y0-side chains run while y's DMA streams in
            lamQ, lamU, lamP = co["Q"]["lam"], co["negU"]["lam"], co["P"]["lam"]
            nc.vector.tensor_scalar(chQ[:], ty0[:], 2.0 * lamQ, -lamQ, ALU.mult, ALU.add)
            _emit_norm_chain(nc, Qacc, chQ, co["Q"], h3, h2)
            nc.vector.tensor_scalar(chU[:], ty0[:], 2.0 * lamU, -lamU, ALU.mult, ALU.add)
            _emit_norm_chain(nc, nUacc, chU, co["negU"], h3, h2)
            vector.wait_ge(s_in, 32)
            nc.vector.tensor_scalar(chP[:], ty[:], 2.0 * lamP, -lamP, ALU.mult, ALU.add)
            _emit_norm_chain(nc, Pacc, chP, co["P"], h3, h2)
            # s0 = sP*Pacc + sU*nUacc (true value Pnc + negUnc)
            if sP > 0 and sU > 0:
                nc.vector.tensor_tensor(s0[:], Pacc[:], nUacc[:], ALU.add)
            elif sP > 0:
                nc.vector.tensor_tensor(s0[:], Pacc[:], nUacc[:], ALU.subtract)
            elif sU > 0:
                nc.vector.tensor_tensor(s0[:], nUacc[:], Pacc[:], ALU.subtract)
            else:
                nc.vector.tensor_tensor(s0[:], Pacc[:], nUacc[:], ALU.add)
            vector.wait_ge(s_act, 1)
            # t2 = c*ly0 + sQ*Qacc + q0
            lm = ops["logmix_p"] if sQ > 0 else ops["logmix_m"]
            nc.vector._custom_dve(lm, out=t2[:], in0=ly0[:], in1=Qacc[:],
                                  s0=cc, s1=co["q0"])
            vector.wait_ge(s_act, 2)
            # z = c*ly - t2
            nc.vector._custom_dve(ops["axmy"], out=z[:], in0=ly[:], in1=t2[:], s0=cc)
            # w = z*y + K0 ; res = +-s0 + w, split in halves to overlap out-DMA
            for lo, hi, inc in ((0, HF, 1), (HF, F_DIM, 1)):
                sl = (slice(None), slice(lo, hi))
                nc.vector._custom_dve(ops["muladd"], out=w[sl], in0=z[sl],
                                      in1=ty[sl], s0=co["K0"])
                if sP < 0 and sU < 0:
                    ins_ = nc.vector.tensor_tensor(res[sl], w[sl], s0[sl], ALU.subtract)
                else:
                    ins_ = nc.vector.tensor_tensor(res[sl], s0[sl], w[sl], ALU.add)
                ins_.then_inc(s_done, inc)

    # Raw Bass skips Bacc's ISA pre-encode; custom-DVE (InstCustomDveAnt)
    # needs .instr bytes populated or walrus fails with "ISA wrong length".
    mybir.codegen_inst_isa_subclasses(nc)
    return nc

# --------------------------------------------------------------------------- #
# entry point
# --------------------------------------------------------------------------- #

_NC_CACHE = {}


def kernel(y, y0, v, w, b, a, c):
    from concourse.bass_utils import run_bass_kernel_spmd

    y = np.ascontiguousarray(y, dtype=np.float32)
    y0 = np.ascontiguousarray(y0, dtype=np.float32)
    co = _gen_coeffs(np.asarray(v), np.asarray(w), np.asarray(b),
                     np.asarray(a).reshape(-1)[0], np.asarray(c).reshape(-1)[0])

    key = (tuple(co["P"]["cp"]), co["P"]["lam"], tuple(co["negU"]["cp"]),
           co["negU"]["lam"], tuple(co["Q"]["cp"]), co["Q"]["lam"],
           co["P"]["sign"], co["negU"]["sign"], co["Q"]["sign"],
           co["K0"], co["q0"], co["c"])
    nc = _NC_CACHE.get(key)
    if nc is None:
        nc = _build_nc(co)
        _NC_CACHE[key] = nc

    yf = y.reshape(-1)
    y0f = y0.reshape(-1)
    eps_arr = np.tile(np.array([[-EPS_PROB, EPS_PROB]], dtype=np.float32),
                      (P_DIM, 1))
    in_maps = []
    for i in range(NCORES):
        sl = slice(i * PER_CORE, (i + 1) * PER_CORE)
        in_maps.append({
            "y_in": yf[sl].reshape(P_DIM, F_DIM),
            "y0_in": y0f[sl].reshape(P_DIM, F_DIM),
            "eps_in": eps_arr,
        })

    res = run_bass_kernel_spmd(nc, in_maps, list(range(NCORES)))
    outs = [np.asarray(r["out"]).reshape(-1) for r in res.results]
    return np.concatenate(outs).reshape(y.shape).astype(np.float32)


# revision 29
# speedup vs baseline: 1.0405x; 1.0405x over previous
"""Trainium2 Bass kernel for nn_NeuralMirrorModule (Bregman divergence loss).

Math: the reference's per-element computation collapses to
    div(y,y0) = S(y) - S(y0) - S'(y0)*(y-y0) + a/2*(y-y0)^2
                + c*(y*(ln ys - ln y0s) - (y-y0))
with S(t) = sum_j v_j * H_j(t) a fixed univariate function of t in [0,1)
determined by the 126 (v,w,b) parameters.  On the host we fit S and S'
with Chebyshev polynomials (fp64); the approximation error sits below
the reference's own fp32 noise floor (~2.9e-6 abs vs absmax ~0.096).
The device evaluates, per element:
    out = P(y) + c*y*ln(y) - U(y0) - (Q(y0) + c*ln(y0s))*y
where P/U/Q are polynomials evaluated by Horner chains in a normalized
variable chi = lam*(2t-1) chosen so the leading coefficient is +-1 -- the
first fused op then consumes 4 coefficients and no chain-start op is
needed.  Horner runs 3 steps per DVE instruction via a custom fused op
(((acc+c1)*x+c2)*x+c3)*x, cody-waite style; the combine tail is 5 more
fused DVE ops; the two logs run on the ACT engine (with the EPS_PROB
clamp folded into ACT as relu(y0-eps)+eps).

Sharding: flat 2M elements -> 8 cores x [128, 2048]; params replicated
(polynomial coefficients baked as instruction immediates).  No
communication.
"""

import numpy as np

NCORES = 8
P_DIM, F_DIM = 128, 2048
PER_CORE = P_DIM * F_DIM          # 262144
DEG_S, DEG_SPU, DEG_SPQ = 7, 9, 10
EPS = 1e-3                        # log clamp eps (activation group 4)
EPS_PROB = 1e-10
NG = 21
ONE_THIRD = 1.0 / 3.0

# --------------------------------------------------------------------------- #
# host-side math: collapse the 126-neuron Bregman potential to polynomials
# --------------------------------------------------------------------------- #

def _act(u, g):
    if g == 0: return u ** 3
    if g == 1: return u ** 2
    if g == 2: return np.sqrt(np.maximum(u, 0.0))
    if g == 3: return np.power(np.maximum(u, 0.0), ONE_THIRD)
    if g == 4: return np.log(np.maximum(u, 0.0) + EPS)
    return np.exp(u)


def _prim(u, ws, g):
    if g == 0: return u ** 4 / (4.0 * ws)
    if g == 1: return u ** 3 / (3.0 * ws)
    if g == 2: return (2.0 / 3.0) * np.power(np.maximum(u, 0.0), 1.5) / ws
    if g == 3: return 0.75 * np.power(np.maximum(u, 0.0), 4.0 / 3.0) / ws
    if g == 4:
        us = np.maximum(u, 0.0) + EPS
        return (us * np.log(us) - us) / ws
    return np.exp(u) / ws


def _norm_chain(C):
    """Lead-1 normalization: poly sum C[k] x^k (x = 2t-1) -> chain in
    chi = lam*x with coefficients cp (cp[d] = +1), overall sign."""
    C = np.asarray(C, dtype=np.float64)
    d = len(C) - 1
    if abs(C[d]) < 1e-12 * max(1e-300, np.abs(C).max()):
        # degenerate leading coeff: nudge it; changes the poly negligibly
        C = C.copy()
        C[d] = 1e-12 * max(1e-300, np.abs(C).max()) or 1e-30
    lam = abs(C[d]) ** (1.0 / d)
    sign = 1.0 if C[d] > 0 else -1.0
    cp = np.array([C[k] / lam ** k for k in range(d + 1)]) * sign
    return dict(lam=float(lam), sign=sign, cp=[float(x) for x in cp])


def _gen_coeffs(v, w, b, a, c):
    """Fit S, S' on [0,1]; return normalized device chains and scalars."""
    import numpy.polynomial.chebyshev as Ch
    import numpy.polynomial.polynomial as Pn

    v = v.astype(np.float64); w = w.astype(np.float64); b = b.astype(np.float64)
    a = float(a); c = float(c)

    def S_of(t):
        out = np.zeros_like(t)
        for g in range(6):
            for j in range(g * NG, (g + 1) * NG):
                u = w[j] * t + b[j]
                if abs(w[j]) < 1e-12:       # degenerate branch of the reference
                    out += v[j] * _act(u, g) * t
                else:
                    out += v[j] * _prim(u, w[j], g)
        return out

    def Sp_of(t):
        out = np.zeros_like(t)
        for g in range(6):
            for j in range(g * NG, (g + 1) * NG):
                out += v[j] * _act(w[j] * t + b[j], g)
        return out

    M = 3000
    xn = np.cos(np.pi * (np.arange(M) + 0.5) / M)
    tn = 0.5 * (xn + 1.0)
    S0 = S_of(np.zeros(1))[0]
    Sv = S_of(tn) - S0
    Spv = Sp_of(tn)
    ps = Ch.cheb2poly(Ch.chebfit(xn, Sv, DEG_S))       # S~ fit, in x = 2t-1
    ppu = Ch.cheb2poly(Ch.chebfit(xn, Spv, DEG_SPU))   # S~' fit for U
    ppq = Ch.cheb2poly(Ch.chebfit(xn, Spv, DEG_SPQ))   # S~' fit for Q

    # P(t) = S~(t) + (a/2)t^2 - c t          [evaluated at y]
    P = ps.copy()
    P[0] += a / 8 - c / 2; P[1] += a / 4 - c / 2; P[2] += a / 8
    # U(t) = S~(t) - t S~'(t) - (a/2)t^2 - c t   [evaluated at y0]
    U = Pn.polysub(ps, Pn.polymul(np.array([0.5, 0.5]), ppu))
    U[0] += -(a / 8) - c / 2; U[1] += -(a / 4) - c / 2; U[2] += -(a / 8)
    # Q(t) = S~'(t) + a t                    [evaluated at y0]
    Q = ppq.copy()
    Q[0] += a / 2; Q[1] += a / 2

    return dict(
        P=_norm_chain(P),
        negU=_norm_chain(-U),
        Q=_norm_chain(Q),
        K0=float(P[0] - U[0]),
        q0=float(Q[0]),
        c=c,
    )

# --------------------------------------------------------------------------- #
# custom DVE ops
# --------------------------------------------------------------------------- #

_OPS_CACHE = {}


def _register_dve_ops():
    """Register fused DVE ops in concourse.dve_ops (runtime append, per the
    documented extension API). Idempotent."""
    if _OPS_CACHE:
        return _OPS_CACHE
    import concourse.dve_ops as D
    from concourse.dve_spec import Spec, Src0, Src1, C0, C1, C2, lower
    from concourse.dve_spec import _has_src1
    from concourse.dve_uop import DveOpSpec

    def make(name, body, ref):
        for op in D.OPS:
            if op.name == name:
                return op
        spec = Spec(body=body, reference=ref)
        shas = {}
        for ver in ("v3", "v4"):
            s = DveOpSpec(name=name, opcode=1, uops=lower(spec, ver=ver),
                          rd1_en=_has_src1(spec))
            shas[ver] = s.sha(ver)
        op = D.DveOp(name, spec, subdim=False, uops_sha=shas)
        D.OPS.append(op)
        row = D._CUSTOM_DVE_ROW_BASE + D.OPS.index(op)
        assert row < 0x20, "custom DVE row overflow"
        D._SUB_OPCODE_FOR_NAME[name] = row
        D.CUSTOM_DVE_SPECS[name] = spec
        return op

    f32 = np.float32
    _OPS_CACHE["h3"] = make(
        "HORNER3_ANT",
        (((Src0 + C0) * Src1 + C1) * Src1 + C2) * Src1,
        lambda in0, in1, s0, s1, imm2: (
            ((((in0.astype(f32) + f32(s0)) * in1 + f32(s1)) * in1 + f32(imm2)) * in1)
        ).astype(f32),
    )
    _OPS_CACHE["h2"] = make(
        "HORNER2_ANT",
        ((Src0 + C0) * Src1 + C1) * Src1,
        lambda in0, in1, s0, s1, imm2: (
            ((in0.astype(f32) + f32(s0)) * in1 + f32(s1)) * in1
        ).astype(f32),
    )
    # t2 = (ly0*c + Qacc) + q0   /  minus variant for sign-flipped Q chains
    _OPS_CACHE["logmix_p"] = make(
        "LOGMIXP_ANT",
        (Src0 * C0 + Src1) + C1,
        lambda in0, in1, s0, s1, imm2: (
            (in0.astype(f32) * f32(s0) + in1) + f32(s1)
        ).astype(f32),
    )
    _OPS_CACHE["logmix_m"] = make(
        "LOGMIXM_ANT",
        (Src0 * C0 - Src1) + C1,
        lambda in0, in1, s0, s1, imm2: (
            (in0.astype(f32) * f32(s0) - in1) + f32(s1)
        ).astype(f32),
    )
    # z = ly*c - t2
    _OPS_CACHE["axmy"] = make(
        "AXMY_ANT",
        Src0 * C0 - Src1,
        lambda in0, in1, s0, s1, imm2: (
            in0.astype(f32) * f32(s0) - in1
        ).astype(f32),
    )
    # w = z*y + K0
    _OPS_CACHE["muladd"] = make(
        "MULADD_ANT",
        Src0 * Src1 + C0,
        lambda in0, in1, s0, s1, imm2: (
            in0.astype(f32) * in1 + f32(s0)
        ).astype(f32),
    )
    return _OPS_CACHE

# --------------------------------------------------------------------------- #
# bass program
# --------------------------------------------------------------------------- #


def _emit_norm_chain(nc, acc, chi, ch, h3, h2, out_slices=None):
    """Lead-1 zero-const Horner: acc <- sign * sum_{k>=1} C[k] x^k, where the
    chain runs in chi (= lam*x) with normalized coeffs ch['cp'] (cp[d]=1).
    First fused op reads chi for both streams (no chain-start op)."""
    import concourse.mybir as mybir
    cp = ch["cp"]
    d = len(cp) - 1
    assert d >= 4
    nc.vector._custom_dve(
        h3, out=acc[:], in0=chi[:], in1=chi[:],
        s0=cp[d - 1], s1=cp[d - 2], imm2=cp[d - 3])
    ks = list(range(d - 4, 0, -1))
    i = 0
    while i < len(ks):
        left = len(ks) - i
        if left >= 3:
            nc.vector._custom_dve(
                h3, out=acc[:], in0=acc[:], in1=chi[:],
                s0=cp[ks[i]], s1=cp[ks[i + 1]], imm2=cp[ks[i + 2]])
            i += 3
        elif left == 2:
            nc.vector._custom_dve(
                h2, out=acc[:], in0=acc[:], in1=chi[:],
                s0=cp[ks[i]], s1=cp[ks[i + 1]])
            i += 2
        else:
            nc.vector.scalar_tensor_tensor(
                acc[:], acc[:], cp[ks[i]], chi[:],
                mybir.AluOpType.add, mybir.AluOpType.mult)
            i += 1


def _build_nc(co, debug_taps=()):
    from contextlib import ExitStack
    import concourse.bass as bass
    import concourse.mybir as mybir

    ops = _register_dve_ops()
    h3, h2 = ops["h3"], ops["h2"]
    f32 = mybir.dt.float32
    ALU = mybir.AluOpType
    AF = mybir.ActivationFunctionType
    HF = F_DIM // 2

    nc = bass.Bass()
    y_in = nc.declare_dram_parameter("y_in", [P_DIM, F_DIM], f32, isOutput=False)
    y0_in = nc.declare_dram_parameter("y0_in", [P_DIM, F_DIM], f32, isOutput=False)
    eps_in = nc.declare_dram_parameter("eps_in", [P_DIM, 2], f32, isOutput=False)
    out_d = nc.declare_dram_parameter("out", [P_DIM, F_DIM], f32, isOutput=True)
    dbg_d = {n: nc.declare_dram_parameter("dbg_" + n, [P_DIM, F_DIM], f32, isOutput=True)
             for n in debug_taps}

    sP, sU, sQ = co["P"]["sign"], co["negU"]["sign"], co["Q"]["sign"]
    cc = co["c"]

    with ExitStack() as es:
        def tile(name):
            return es.enter_context(nc.sbuf_tensor(name, [P_DIM, F_DIM], f32))

        ty, ty0 = tile("ty"), tile("ty0")
        chP, chU, chQ, tr = tile("chP"), tile("chU"), tile("chQ"), tile("tr")
        ly, ly0 = tile("ly"), tile("ly0")
        Pacc, nUacc, Qacc = tile("Pacc"), tile("nUacc"), tile("Qacc")
        t2, z, w, s0, res = tile("t2"), tile("z"), tile("w"), tile("s0"), tile("res")
        bias_t = es.enter_context(nc.sbuf_tensor("bias_t", [P_DIM, 2], f32))

        s_in = es.enter_context(nc.semaphore("s_in"))
        s_ing = es.enter_context(nc.semaphore("s_ing"))
        s_act = es.enter_context(nc.semaphore("s_act"))
        s_done = es.enter_context(nc.semaphore("s_done"))
        s_out = es.enter_context(nc.semaphore("s_out"))
        s_out2 = es.enter_context(nc.semaphore("s_out2"))
        block = es.enter_context(nc.Block(no_gpsimd_drain=True))

        tiles_by_name = dict(ty=ty, ty0=ty0, chP=chP, chU=chU, chQ=chQ, tr=tr,
                             ly=ly, ly0=ly0, Pacc=Pacc, nUacc=nUacc, Qacc=Qacc,
                             t2=t2, z=z, w=w, s0=s0, res=res)

        @block.sync
        def _(sync):
            # single whole-tile DMAs: one InstDMACopy already fans out across
            # all 16 SDMA engines; splitting across rings just contends
            sync.dma_start(out=ty0[:], in_=y0_in[:]).then_inc(s_in, 16)
            sync.dma_start(out=ty[:], in_=y_in[:]).then_inc(s_in, 16)
            sync.wait_ge(s_done, 1)
            # no completion wait: the Block-exit Drain on SP drains its HWDGE
            # ring, which implies the output DMA has landed
            sync.dma_start(out=out_d[:, :HF], in_=res[:, :HF]).then_inc(s_out, 16)
            for n in debug_taps:
                sync.dma_start(out=dbg_d[n][:], in_=tiles_by_name[n][:]).then_inc(s_out, 16)

        @block.scalar
        def _(scalar):
            # eps biases ride ACT's own HWDGE ring (tiny)
            scalar.dma_start(out=bias_t[:], in_=eps_in[:]).then_inc(s_ing, 16)
            scalar.wait_ge(s_in, 16)
            scalar.wait_ge(s_ing, 16)
            # ln(max(t, eps)) == ln(relu(t - eps) + eps), all on ACT
            nc.scalar.activation(tr[:], ty0[:], AF.Relu, bias=bias_t[:, 0:1])
            nc.scalar.activation(ly0[:], tr[:], AF.Ln, bias=bias_t[:, 1:2]).then_inc(s_act, 1)
            scalar.wait_ge(s_in, 32)
            nc.scalar.activation(tr[:], ty[:], AF.Relu, bias=bias_t[:, 0:1])
            nc.scalar.activation(ly[:], tr[:], AF.Ln, bias=bias_t[:, 1:2]).then_inc(s_act, 1)
            scalar.wait_ge(s_done, 2)
            scalar.dma_start(out=out_d[:, HF:], in_=res[:, HF:]).then_inc(s_out2, 16)

        @block.vector
        def _(vector):
            vector.wait_ge(s_in, 16)
            # chi variables; y0-side chains run while y's DMA streams in
            lamQ, lamU, lamP = co["Q"]["lam"], co["negU"]["lam"], co["P"]["lam"]
            nc.vector.tensor_scalar(chQ[:], ty0[:], 2.0 * lamQ, -lamQ, ALU.mult, ALU.add)
            _emit_norm_chain(nc, Qacc, chQ, co["Q"], h3, h2)
            nc.vector.tensor_scalar(chU[:], ty0[:], 2.0 * lamU, -lamU, ALU.mult, ALU.add)
            _emit_norm_chain(nc, nUacc, chU, co["negU"], h3, h2)
            vector.wait_ge(s_in, 32)
            nc.vector.tensor_scalar(chP[:], ty[:], 2.0 * lamP, -lamP, ALU.mult, ALU.add)
            _emit_norm_chain(nc, Pacc, chP, co["P"], h3, h2)
            # s0 = sP*Pacc + sU*nUacc (true value Pnc + negUnc)
            if sP > 0 and sU > 0:
                nc.vector.tensor_tensor(s0[:], Pacc[:], nUacc[:], ALU.add)
            elif sP > 0:
                nc.vector.tensor_tensor(s0[:], Pacc[:], nUacc[:], ALU.subtract)
            elif sU > 0:
                nc.vector.tensor_tensor(s0[:], nUacc[:], Pacc[:], ALU.subtract)
            else:
                nc.vector.tensor_tensor(s0[:], Pacc[:], nUacc[:], ALU.add)
            vector.wait_ge(s_act, 1)
            # t2 = c*ly0 + sQ*Qacc + q0
            lm = ops["logmix_p"] if sQ > 0 else ops["logmix_m"]
            nc.vector._custom_dve(lm, out=t2[:], in0=ly0[:], in1=Qacc[:],
                                  s0=cc, s1=co["q0"])
            vector.wait_ge(s_act, 2)
            # z = c*ly - t2
            nc.vector._custom_dve(ops["axmy"], out=z[:], in0=ly[:], in1=t2[:], s0=cc)
            # w = z*y + K0 ; res = +-s0 + w, split in halves to overlap out-DMA
            for lo, hi, inc in ((0, HF, 1), (HF, F_DIM, 1)):
                sl = (slice(None), slice(lo, hi))
                nc.vector._custom_dve(ops["muladd"], out=w[sl], in0=z[sl],
                                      in1=ty[sl], s0=co["K0"])
                if sP < 0 and sU < 0:
                    ins_ = nc.vector.tensor_tensor(res[sl], w[sl], s0[sl], ALU.subtract)
                else:
                    ins_ = nc.vector.tensor_tensor(res[sl], s0[sl], w[sl], ALU.add)
                ins_.then_inc(s_done, inc)

    # Raw Bass skips Bacc's ISA pre-encode; custom-DVE (InstCustomDveAnt)
    # needs .instr bytes populated or walrus fails with "ISA wrong length".
    mybir.codegen_inst_isa_subclasses(nc)
    return nc

# --------------------------------------------------------------------------- #
# entry point
# --------------------------------------------------------------------------- #

_NC_CACHE = {}


def kernel(y, y0, v, w, b, a, c):
    from concourse.bass_utils import run_bass_kernel_spmd

    y = np.ascontiguousarray(y, dtype=np.float32)
    y0 = np.ascontiguousarray(y0, dtype=np.float32)
    co = _gen_coeffs(np.asarray(v), np.asarray(w), np.asarray(b),
                     np.asarray(a).reshape(-1)[0], np.asarray(c).reshape(-1)[0])

    key = (tuple(co["P"]["cp"]), co["P"]["lam"], tuple(co["negU"]["cp"]),
           co["negU"]["lam"], tuple(co["Q"]["cp"]), co["Q"]["lam"],
           co["P"]["sign"], co["negU"]["sign"], co["Q"]["sign"],
           co["K0"], co["q0"], co["c"])
    nc = _NC_CACHE.get(key)
    if nc is None:
        nc = _build_nc(co)
        _NC_CACHE[key] = nc

    yf = y.reshape(-1)
    y0f = y0.reshape(-1)
    eps_arr = np.tile(np.array([[-EPS_PROB, EPS_PROB]], dtype=np.float32),
                      (P_DIM, 1))
    in_maps = []
    for i in range(NCORES):
        sl = slice(i * PER_CORE, (i + 1) * PER_CORE)
        in_maps.append({
            "y_in": yf[sl].reshape(P_DIM, F_DIM),
            "y0_in": y0f[sl].reshape(P_DIM, F_DIM),
            "eps_in": eps_arr,
        })

    res = run_bass_kernel_spmd(nc, in_maps, list(range(NCORES)))
    outs = [np.asarray(r["out"]).reshape(-1) for r in res.results]
    return np.concatenate(outs).reshape(y.shape).astype(np.float32)
